# revision 1
# baseline (speedup 1.0000x reference)
"""CPQuadRankLayer Trainium2 kernel, fully host-prepacked layouts.

Math (per node n, batch b):
  P[b,c,r]  = sum_i x[b,n,c,i] * factors[c,n,r,i]
  p         = P / sqrt(mean_r P^2 + eps)
  merged    = p0*p1*p2*p3 * gain[n]
  out[b,o]  = sum_r merged[b,r] * factor_out[n,r,o] + mean_c x[b,n,c,o]

Distribution: nodes sharded 1024 -> 8 cores x 128 nodes (node-
independent: no replication, no collectives). All tensors are repacked
on the host so every DMA runs full-width with >=2KiB contiguous runs
and the contraction dims land directly on SBUF partitions (no on-chip
transposes of x or factors; only the tiny per-pair merged transpose
remains on the PE). The second matmul produces transposed output
[o, b] so the residual is applied in the same space; the packed output
is unpacked on the host.
"""

import numpy as np

B = 64
N = 1024
C = 4
D = 128
R = 64
NCORES = 8
NS = N // NCORES  # nodes per core (128)
G = 16  # nodes per group
NH = NS // 2  # node pairs per core
GH = G // 2  # node pairs per group
NG = NS // G  # groups per core (8)
OCT = NS // 8  # octets per core (16)
QUAD = NS // 4  # quads per core (32)
EPS = 1e-6

_CACHE = {}


def _build_nc(repeat=1):
    import concourse.bacc as bacc
    import concourse.tile as tile
    import concourse.mybir as mybir
    from concourse.masks import make_identity

    dt = mybir.dt.float32
    Act = mybir.ActivationFunctionType

    nc = bacc.Bacc()
    # x pre-packed: [group, c, i, (node16, b)] -> 4KiB runs
    xp = nc.declare_dram_parameter("xp", [NG, C, D, 1024], dt, isOutput=False)
    # factors pre-packed: [c, group, i, (node16, r)] -> 4KiB runs
    f = nc.declare_dram_parameter("factors_t", [C, NG, D, 1024], dt, isOutput=False)
    # factor_out pre-packed: [octet, r, (node8, o)] -> 4KiB runs
    fo = nc.declare_dram_parameter("factor_out_t", [OCT, R, 8 * D], dt, isOutput=False)
    gain = nc.declare_dram_parameter("gain", [NS, 1], dt, isOutput=False)
    # packed output: [group, o, (gh, g2, b)]; host unpacks
    out = nc.declare_dram_parameter("out_t", [NG, 128, GH * D], dt, isOutput=True)

    xp_r = xp.rearrange("g c i w -> i g c w")
    f_r = f.rearrange("c g i w -> i g c w")
    fo_r = fo.rearrange("u r w -> r u w")

    with tile.TileContext(nc) as tc:
        with (
            tc.tile_pool(name="consts", bufs=1) as consts,
            tc.tile_pool(name="xpool", bufs=3) as xpool,
            tc.tile_pool(name="fpool", bufs=3) as fpool,
            tc.tile_pool(name="fopool", bufs=3) as fopool,
            tc.tile_pool(name="opool", bufs=2) as opool,
            tc.tile_pool(name="ppool", bufs=2) as ppool,
            tc.tile_pool(name="sqpool", bufs=2) as sqpool,
            tc.tile_pool(name="rpool", bufs=2) as rpool,
            tc.tile_pool(name="work", bufs=3) as work,
            tc.tile_pool(name="small", bufs=4) as small,
            tc.tile_pool(name="trps", bufs=1, space="PSUM") as trps,
            tc.tile_pool(name="pps", bufs=3, space="PSUM") as pps,
            tc.tile_pool(name="mtps", bufs=2, space="PSUM") as mtps,
            tc.tile_pool(name="ops", bufs=2, space="PSUM") as ops,
        ):
            identity = consts.tile([128, 128], dt)
            make_identity(nc, identity)
            eps_t = consts.tile([128, 1], dt)
            nc.vector.memset(eps_t, EPS)

            # gpair[p, h] = gain[2h + (p >= 64)] via two K=1 outer products
            ones1 = consts.tile([1, 128], dt)
            nc.vector.memset(ones1, 1.0)
            g1 = consts.tile([1, NS], dt)
            nc.sync.dma_start(out=g1, in_=gain.rearrange("n o -> o n"))
            g1v = g1.rearrange("o (h g2) -> o h g2", g2=2)
            gpp = trps.tile([128, NH], dt, tag="tr")
            nc.tensor.matmul(gpp[0:64, :], lhsT=ones1[:, 0:64], rhs=g1v[:, :, 0])
            nc.tensor.matmul(gpp[64:128, :], lhsT=ones1[:, 0:64], rhs=g1v[:, :, 1])
            gpair = consts.tile([128, NH], dt)
            nc.any.tensor_copy(gpair, gpp)

            def load(gi):
                st = {}
                xt_g = xpool.tile([128, C, 1024], dt, tag="xt_g")
                nc.sync.dma_start(out=xt_g, in_=xp_r[:, gi])
                f_t = fpool.tile([128, C, 1024], dt, tag="f_t")
                nc.scalar.dma_start(out=f_t, in_=f_r[:, gi])
                fo_t = fopool.tile([R, 2, 8 * D], dt, tag="fo_t")
                nc.scalar.dma_start(out=fo_t, in_=fo_r[:, 2 * gi : 2 * gi + 2])
                st["x"], st["f"], st["fo"] = xt_g, f_t, fo_t
                return st

            def phase1(gi, st):
                ppall = ppool.tile([128, GH, C, R], dt, tag="ppall")
                st["pp"] = ppall
                for ghp in range(0, GH, 2):
                    pp = pps.tile([128, 2, C, R], dt, tag="pp")
                    for dg in range(2):
                        gh = ghp + dg
                        for c in range(C):
                            for g2 in range(2):
                                j = 2 * gh + g2
                                nc.tensor.matmul(
                                    pp[64 * g2 : 64 * g2 + 64, dg, c, :],
                                    lhsT=st["x"][:, c, 64 * j : 64 * j + 64],
                                    rhs=st["f"][:, c, 64 * j : 64 * j + 64],
                                )
                    nc.scalar.copy(out=st["pp"][:, ghp : ghp + 2], in_=pp)

            def stats(gi, st):
                h0 = gi * GH
                ppall = st["pp"]
                sq = sqpool.tile([128, GH, C, R], dt, tag="sq")
                nc.scalar.activation(out=sq, in_=ppall, func=Act.Square)
                ssq = small.tile([128, GH * C], dt, tag="ssq")
                nc.vector.reduce_sum(
                    out=ssq,
                    in_=sq.rearrange("p gh c r -> p (gh c) r"),
                    axis=mybir.AxisListType.X,
                )
                rms = small.tile([128, GH * C], dt, tag="rms")
                nc.scalar.activation(
                    out=rms, in_=ssq, func=Act.Sqrt, bias=eps_t, scale=1.0 / R
                )
                rstd = small.tile([128, GH, C], dt, tag="rstd")
                nc.vector.reciprocal(
                    out=rstd, in_=rms.rearrange("p (gh c) -> p gh c", c=C)
                )
                sa = small.tile([128, GH], dt, tag="sa")
                nc.vector.tensor_mul(sa, rstd[:, :, 0], rstd[:, :, 1])
                sb = small.tile([128, GH], dt, tag="sb")
                nc.vector.tensor_mul(sb, rstd[:, :, 2], rstd[:, :, 3])
                sab = small.tile([128, GH], dt, tag="sab")
                nc.vector.tensor_mul(sab, sa, sb)
                scl2 = small.tile([128, GH], dt, tag="scl2")
                nc.vector.tensor_mul(scl2, sab, gpair[:, h0 : h0 + GH])

                rt1 = rpool.tile([128, 1024], dt, tag="ra")
                nc.vector.tensor_add(rt1, st["x"][:, 0], st["x"][:, 1])
                rt2 = rpool.tile([128, 1024], dt, tag="rb")
                nc.vector.tensor_add(rt2, st["x"][:, 2], st["x"][:, 3])
                rts = rpool.tile([128, 1024], dt, tag="rb")
                nc.vector.tensor_add(rts, rt1, rt2)
                xq = rpool.tile([128, 1024], dt, tag="ra")
                nc.vector.tensor_scalar_mul(out=xq, in0=rts, scalar1=0.25)
                st["xq"] = xq

                m01 = work.tile([128, GH, R], dt, tag="m01")
                nc.vector.tensor_mul(m01, ppall[:, :, 0, :], ppall[:, :, 1, :])
                m23 = work.tile([128, GH, R], dt, tag="m23")
                nc.vector.tensor_mul(m23, ppall[:, :, 2, :], ppall[:, :, 3, :])
                mgall = work.tile([128, GH, R], dt, tag="mgall")
                nc.vector.tensor_mul(mgall, m01, m23)
                scl2b = scl2.unsqueeze(2).broadcast_to([128, GH, R])
                nc.vector.tensor_mul(mgall, mgall, scl2b)
                st["mg"] = mgall

            def phase2(gi, st):
                o_t = opool.tile([128, GH, D], dt, tag="o_t")
                for ghp in range(0, GH, 2):
                    op = ops.tile([128, 2, D], dt, tag="op")
                    for dg in range(2):
                        gh = ghp + dg
                        mtp = mtps.tile([64, 128], dt, tag="mtp")
                        nc.tensor.matmul(mtp, lhsT=st["mg"][:, gh, :], rhs=identity)
                        mt = work.tile([64, 128], dt, tag="mt")
                        nc.scalar.copy(out=mt, in_=mtp)
                        for g2 in range(2):
                            j = 2 * gh + g2
                            u8, j8 = j // 8, j % 8
                            nc.tensor.matmul(
                                op[:, dg, 64 * g2 : 64 * g2 + 64],
                                lhsT=st["fo"][:, u8, 128 * j8 : 128 * j8 + 128],
                                rhs=mt[:, 64 * g2 : 64 * g2 + 64],
                            )
                    nc.vector.tensor_add(
                        o_t[:, ghp : ghp + 2, :],
                        op,
                        st["xq"][:, 128 * ghp : 128 * ghp + 256].rearrange(
                            "p (two o) -> p two o", two=2
                        ),
                    )
                nc.sync.dma_start(
                    out=out[gi], in_=o_t.rearrange("p gh o -> p (gh o)")
                )

            def emit_all_groups():
                # software pipeline: phase1 of group gi+1 is emitted in
                # the shadow of group gi's stats chain so the PE stream
                # never drains between groups
                # prefetch distance 2: group gi+2's loads are issued a
                # full group-time before its matmuls need them
                prev = load(0)
                phase1(0, prev)
                nxt = load(1)
                for gi in range(NG):
                    nxt2 = load(gi + 2) if gi + 2 < NG else None
                    stats(gi, prev)
                    if nxt is not None:
                        phase1(gi + 1, nxt)
                    phase2(gi, prev)
                    prev, nxt = nxt, nxt2

            if repeat > 1:
                with tc.For_i(0, repeat, 1):
                    emit_all_groups()
            else:
                emit_all_groups()

    nc.compile()
    return nc


def _get_nc(repeat=1):
    key = ("nc", repeat)
    if key not in _CACHE:
        _CACHE[key] = _build_nc(repeat)
    return _CACHE[key]


def _pack_x(x):
    # [B, N, C, D] -> [N//16, C, D, 1024] ; n = g*16 + j, col = j*64 + b
    a = x.reshape(B, N // 16, 16, C, D)
    a = np.transpose(a, (1, 3, 4, 2, 0))  # [g, c, i, j, b]
    return np.ascontiguousarray(a.reshape(N // 16, C, D, 1024))


def _pack_factors(factors):
    # [4, N, R, D] -> [C, N//16, D, 1024]
    f = factors.reshape(C, N // 16, 16, R, D)
    f = np.transpose(f, (0, 1, 4, 2, 3))  # [c, g, i, j, r]
    return np.ascontiguousarray(f.reshape(C, N // 16, D, 1024))


def _pack_factor_out(factor_out):
    # [N, R, D] -> [N//8, R, 8*D]
    q = factor_out.reshape(N // 8, 8, R, D)
    q = np.transpose(q, (0, 2, 1, 3))  # [oct, r, node8, o]
    return np.ascontiguousarray(q.reshape(N // 8, R, 8 * D))


def _unpack_out(res_t):
    # [NG, 128(o), GH*D] with col = gh*128 + g2*64 + b -> [B, NS, D]
    a = res_t.reshape(NG, 128, GH, 2, 64)  # [gi, o, gh, g2, b]
    a = np.transpose(a, (4, 0, 2, 3, 1))  # [b, gi, gh, g2, o]
    return np.ascontiguousarray(a.reshape(64, NS, D))


def kernel(x, factors, factor_out, gain):
    from concourse.bass_utils import run_bass_kernel_spmd

    nc = _get_nc()
    x_packed = _pack_x(np.asarray(x))
    f_packed = _pack_factors(np.asarray(factors))
    fo_packed = _pack_factor_out(np.asarray(factor_out))
    in_maps = []
    for k in range(NCORES):
        lo, hi = k * NS, (k + 1) * NS
        in_maps.append(
            {
                "xp": np.ascontiguousarray(x_packed[k * NG : (k + 1) * NG]),
                "factors_t": np.ascontiguousarray(f_packed[:, k * NG : (k + 1) * NG]),
                "factor_out_t": np.ascontiguousarray(
                    fo_packed[k * OCT : (k + 1) * OCT]
                ),
                "gain": np.ascontiguousarray(gain[lo:hi]),
            }
        )
    res = run_bass_kernel_spmd(nc, in_maps, core_ids=list(range(NCORES)))
    return np.concatenate(
        [_unpack_out(res.results[k]["out_t"]) for k in range(NCORES)], axis=1
    )

